# revision 3
# baseline (speedup 1.0000x reference)
"""GAT (bipartite GATConv + mean-pool + 2 FC) on 8 Trainium2 NeuronCores.

Strategy: shard destination nodes across the 8 cores (each core owns N/8 dst
nodes and all edges pointing at them) so the segment softmax is fully local to
a core — no collectives needed.  Per core:

  Phase A: dense matmuls build a node table  row[n] = [h_s[n] (36) | a_s[n] (3)]
           in core-local DRAM (h_s = x_s @ W, a_s folded as x_s @ (W*att_src)),
           plus per-dst-node a_t values kept in SBUF.
  Phase B: dst nodes are processed in tiles of 128 (one node per partition,
           nodes degree-sorted so tiles have uniform run lengths).  Each node's
           incoming edges occupy L slots along its partition's free dimension;
           slot data arrives via indirect DMA row gathers from the table.
           The segment softmax (skipping the max-subtraction: logits are
           bounded, exp is safe in fp32) and the weighted message sum are
           strided DVE/ACT ops along each partition's run.  A one-hot matmul
           pools relu(out)·W2 and node counts into per-batch partials.

Host work is limited to index manipulation (edge sorting / padding / layout),
weight folding, and the final unsharding reduction of 8 x [128,2] partials.
"""

import numpy as np

import concourse.bacc as bacc
import concourse.tile as tile
from concourse import mybir
from concourse.bass import IndirectOffsetOnAxis
from concourse.bass_utils import run_bass_kernel_spmd

F32 = mybir.dt.float32
I32 = mybir.dt.int32

N_CORES = 8
P = 128
HEADS = 3
CH = 12
HC = HEADS * CH          # 36
ROW = HC + 4             # table row: 36 h | 3 a_s | 1 pad  (40 f32 = 160B)
PAD_A = -300.0           # pad-slot a_s value: exp(0.2*-300) = e^-60 ~ 0
NEG_SLOPE = 0.2

_nc_cache = {}


def _build_nc(in_dim, n_src_tiles, n_dst_tiles, L_list, slot_tot, n_xt_cols):
    """Build the SPMD Bass program (identical for all cores)."""
    key = (in_dim, n_src_tiles, n_dst_tiles, tuple(L_list), slot_tot, n_xt_cols)
    if key in _nc_cache:
        return _nc_cache[key]

    table_rows = n_src_tiles * P + 1          # +1 pad row
    pad_row_idx = n_src_tiles * P
    xs_cols = n_src_tiles * P

    nc = bacc.Bacc("TRN2", target_bir_lowering=False, debug=False)
    d_xs = nc.dram_tensor("xs_t", [in_dim, xs_cols], F32, kind="ExternalInput")
    d_xt = nc.dram_tensor("xt_t", [in_dim, n_xt_cols], F32, kind="ExternalInput")
    d_idx = nc.dram_tensor("idxs", [P, slot_tot], I32, kind="ExternalInput")
    d_bc = nc.dram_tensor("bidcnt", [P, n_dst_tiles * 2], F32, kind="ExternalInput")
    d_wf = nc.dram_tensor("wfold", [in_dim, ROW], F32, kind="ExternalInput")
    d_wt = nc.dram_tensor("wat", [in_dim, 4], F32, kind="ExternalInput")
    d_w2 = nc.dram_tensor("w2b", [P, HC], F32, kind="ExternalInput")
    d_bb = nc.dram_tensor("biasb", [P, HC], F32, kind="ExternalInput")
    d_pr = nc.dram_tensor("padrow", [1, ROW], F32, kind="ExternalInput")
    d_q = nc.dram_tensor("q_out", [P, 2], F32, kind="ExternalOutput")

    with tile.TileContext(nc) as tc:
        with tc.tile_pool(name="const", bufs=1) as cpool, \
             tc.tile_pool(name="dram", bufs=1, space="DRAM") as dpool, \
             tc.tile_pool(name="xload", bufs=2) as xpool, \
             tc.tile_pool(name="tabout", bufs=3) as topool, \
             tc.tile_pool(name="gat", bufs=3) as gpool, \
             tc.tile_pool(name="work", bufs=2) as wpool, \
             tc.tile_pool(name="psA", bufs=3, space="PSUM") as psA, \
             tc.tile_pool(name="psB", bufs=2, space="PSUM") as psB:

            table = dpool.tile([table_rows, ROW], F32)

            # ---- constants into SBUF ----
            t_wf = cpool.tile([in_dim, ROW], F32)
            nc.sync.dma_start(t_wf[:], d_wf[:])
            t_wt = cpool.tile([in_dim, 4], F32)
            nc.sync.dma_start(t_wt[:], d_wt[:])
            t_w2 = cpool.tile([P, HC], F32)
            nc.sync.dma_start(t_w2[:], d_w2[:])
            t_bb = cpool.tile([P, HC], F32)
            nc.sync.dma_start(t_bb[:], d_bb[:])
            t_pr = cpool.tile([1, ROW], F32)
            nc.sync.dma_start(t_pr[:], d_pr[:])
            t_idx = cpool.tile([P, slot_tot], I32)
            nc.sync.dma_start(t_idx[:], d_idx[:])
            t_bc = cpool.tile([P, n_dst_tiles * 2], F32)
            nc.sync.dma_start(t_bc[:], d_bc[:])
            t_xt = cpool.tile([in_dim, n_xt_cols], F32)
            nc.sync.dma_start(t_xt[:], d_xt[:])

            t_iota_i = cpool.tile([P, P], I32)
            nc.gpsimd.iota(t_iota_i[:], pattern=[[1, P]], base=0, channel_multiplier=0)
            t_iota = cpool.tile([P, P], F32)
            nc.vector.tensor_copy(t_iota[:], t_iota_i[:])

            t_qacc = cpool.tile([P, 2], F32)
            nc.vector.memset(t_qacc[:], 0.0)

            # ---- phase A: node table (h_s | a_s), batched 4 tiles / psum ----
            XB = 8  # src tiles per x-chunk load
            for blk in range(0, n_src_tiles, XB):
                nb = min(XB, n_src_tiles - blk)
                xs_sb = xpool.tile([in_dim, XB * P], F32, tag="xs")
                nc.sync.dma_start(
                    xs_sb[:, : nb * P], d_xs[:, blk * P:(blk + nb) * P])
                for g in range(0, nb, 4):
                    ng = min(4, nb - g)
                    ps = psA.tile([P, 4 * ROW], F32, space="PSUM", tag="psa")
                    for j in range(ng):
                        nc.tensor.matmul(
                            ps[:, j * ROW:(j + 1) * ROW],
                            lhsT=xs_sb[:, (g + j) * P:(g + j + 1) * P],
                            rhs=t_wf[:],
                            start=True, stop=True)
                    ob = topool.tile([P, 4 * ROW], F32, tag="tab")
                    nc.vector.tensor_copy(ob[:, : ng * ROW], ps[:, : ng * ROW])
                    # rows r = (blk+g)*128 .. +ng*128; row(p, j) = base + j*128 + p
                    base = (blk + g) * P
                    out_ap = table[:][base:base + ng * P, :]
                    # iterate (p, j, col) to match sbuf order
                    out_ap = out_ap.rearrange("(j p) c -> p j c", p=P)
                    nc.sync.dma_start(
                        out_ap,
                        ob[:, : ng * ROW].rearrange("p (j c) -> p j c", c=ROW))
            # pad row
            nc.sync.dma_start(table[:][pad_row_idx:pad_row_idx + 1, :], t_pr[:])

            # ---- phase A2: a_t per dst tile -> resident SBUF ----
            t_at = cpool.tile([P, n_dst_tiles * 4], F32)
            for t in range(n_dst_tiles):
                ps = psA.tile([P, 4], F32, space="PSUM", tag="psat")
                nc.tensor.matmul(
                    ps[:], lhsT=t_xt[:, t * P:(t + 1) * P], rhs=t_wt[:],
                    start=True, stop=True)
                nc.scalar.copy(t_at[:, t * 4:(t + 1) * 4], ps[:])

            # ---- phase B ----
            off = 0
            for t in range(n_dst_tiles):
                L = L_list[t]
                g = gpool.tile([P, L * ROW], F32, tag="G")
                for s in range(L):
                    nc.gpsimd.indirect_dma_start(
                        out=g[:, s * ROW:(s + 1) * ROW],
                        out_offset=None,
                        in_=table[:],
                        in_offset=IndirectOffsetOnAxis(
                            ap=t_idx[:, off + s:off + s + 1], axis=0),
                    )
                off += L
                g3 = g[:].rearrange("p (l c) -> p l c", c=ROW)

                # logits l = a_s + a_t  (per head, a_t per-partition scalar)
                tT = wpool.tile([P, L * HEADS], F32, tag="T")
                T3 = tT[:].rearrange("p (l h) -> p l h", h=HEADS)
                for h in range(HEADS):
                    nc.vector.tensor_scalar_add(
                        T3[:, :, h], g3[:, :, HC + h], t_at[:, t * 4 + h:t * 4 + h + 1])
                # e = exp(leaky_relu(l))
                tE = wpool.tile([P, L * HEADS], F32, tag="E")
                nc.vector.tensor_scalar_mul(tE[:], tT[:], NEG_SLOPE)
                nc.vector.tensor_tensor(
                    out=tE[:], in0=tE[:], in1=tT[:], op=mybir.AluOpType.max)
                nc.scalar.activation(tE[:], tE[:], mybir.ActivationFunctionType.Exp)
                E3 = tE[:].rearrange("p (l h) -> p l h", h=HEADS)

                # denom + reciprocal
                t_den = wpool.tile([P, HEADS], F32, tag="den")
                nc.vector.tensor_reduce(
                    out=t_den[:], in_=E3.transpose([0, 2, 1]),
                    axis=mybir.AxisListType.X, op=mybir.AluOpType.add)
                nc.vector.tensor_scalar_max(t_den[:], t_den[:], 1e-30)
                t_rec = wpool.tile([P, HEADS], F32, tag="rec")
                nc.vector.reciprocal(t_rec[:], t_den[:])

                # weighted message sum U = sum_l e * h
                tM = wpool.tile([P, L * HC], F32, tag="M")
                M3 = tM[:].rearrange("p (l j) -> p l j", j=HC)
                e_b = E3.unsqueeze(3).to_broadcast((P, L, HEADS, CH))
                nc.vector.tensor_tensor(
                    out=M3[:], in0=g3[:, :, 0:HC], in1=e_b, op=mybir.AluOpType.mult)
                tU = wpool.tile([P, HC], F32, tag="U")
                nc.vector.tensor_reduce(
                    out=tU[:], in_=M3.transpose([0, 2, 1]),
                    axis=mybir.AxisListType.X, op=mybir.AluOpType.add)

                # out = relu(U / denom + bias)
                tV = wpool.tile([P, HC], F32, tag="V")
                rec_b = t_rec[:].unsqueeze(2).to_broadcast((P, HEADS, CH))
                nc.vector.tensor_tensor(
                    out=tV[:].rearrange("p (h c) -> p h c", c=CH),
                    in0=tU[:].rearrange("p (h c) -> p h c", c=CH),
                    in1=rec_b, op=mybir.AluOpType.mult)
                nc.vector.tensor_tensor(
                    out=tV[:], in0=tV[:], in1=t_bb[:], op=mybir.AluOpType.add)
                nc.scalar.activation(tV[:], tV[:], mybir.ActivationFunctionType.Relu)

                # rv = sum(V * W2); RV = [rv | cnt]
                tR = wpool.tile([P, HC], F32, tag="R")
                nc.vector.tensor_tensor(
                    out=tR[:], in0=tV[:], in1=t_w2[:], op=mybir.AluOpType.mult)
                tRV = wpool.tile([P, 2], F32, tag="RV")
                nc.vector.tensor_reduce(
                    out=tRV[:, 0:1], in_=tR[:], axis=mybir.AxisListType.X,
                    op=mybir.AluOpType.add)
                nc.scalar.copy(tRV[:, 1:2], t_bc[:, 2 * t + 1:2 * t + 2])

                # pool into batches: q += onehot(bid)^T @ RV
                t_oh = wpool.tile([P, P], F32, tag="oh")
                nc.vector.tensor_scalar(
                    out=t_oh[:], in0=t_iota[:], scalar1=t_bc[:, 2 * t:2 * t + 1],
                    scalar2=None, op0=mybir.AluOpType.is_equal)
                ps_q = psB.tile([P, 2], F32, space="PSUM", tag="q")
                nc.tensor.matmul(ps_q[:], lhsT=t_oh[:], rhs=tRV[:],
                                 start=True, stop=True)
                nc.vector.tensor_tensor(
                    out=t_qacc[:], in0=t_qacc[:], in1=ps_q[:],
                    op=mybir.AluOpType.add)

            nc.sync.dma_start(d_q[:], t_qacc[:])
    nc.finalize()
    _nc_cache[key] = nc
    return nc


def kernel(**inputs):
    x_s = np.asarray(inputs["x_s"], np.float32)
    x_t = np.asarray(inputs["x_t"], np.float32)
    edge_index = np.asarray(inputs["edge_index"])
    x_s_batch = np.asarray(inputs["x_s_batch"]).astype(np.int64)
    W = np.asarray(inputs["W"], np.float32)
    att_src = np.asarray(inputs["att_src"], np.float32)
    att_dst = np.asarray(inputs["att_dst"], np.float32)
    bias = np.asarray(inputs["bias"], np.float32)
    fc1_w = np.asarray(inputs["fc1_w"], np.float32)
    fc1_b = np.asarray(inputs["fc1_b"], np.float32)
    fc3_w = np.asarray(inputs["fc3_w"], np.float32)
    fc3_b = np.asarray(inputs["fc3_b"], np.float32)

    n_nodes, in_dim = x_s.shape
    src = edge_index[0].astype(np.int64)
    dst = edge_index[1].astype(np.int64)

    # ---- host: edge bucketing by destination (layout prep only) ----
    deg = np.bincount(dst, minlength=n_nodes)
    order = np.argsort(-deg, kind="stable")      # nodes by degree desc
    # round-robin deal over cores: core c gets ranks c, c+8, ...
    nodes_per_core = (n_nodes + N_CORES - 1) // N_CORES
    n_dst_tiles = (nodes_per_core + P - 1) // P
    # per-tile run length: max degree in the global rank band of the tile
    L_list = []
    for t in range(n_dst_tiles):
        r0 = t * P * N_CORES
        L_list.append(max(1, int(deg[order[min(r0, n_nodes - 1)]])))
    slot_tot = int(np.sum(L_list))

    n_src_tiles = (n_nodes + P - 1) // P
    pad_row_idx = n_src_tiles * P
    xs_cols = n_src_tiles * P
    n_xt_cols = n_dst_tiles * P

    # edges sorted by dst -> per-node contiguous src runs
    e_order = np.argsort(dst, kind="stable")
    src_sorted = src[e_order].astype(np.int32)
    starts = np.searchsorted(dst[e_order], np.arange(n_nodes))
    ends = np.searchsorted(dst[e_order], np.arange(n_nodes) + 1)

    xs_t = np.zeros((in_dim, xs_cols), np.float32)
    xs_t[:, :n_nodes] = x_s.T

    # fold weights (host weight prep)
    wa_t = np.einsum("khc,hc->kh", W.reshape(in_dim, HEADS, CH), att_dst).astype(np.float32)
    wa_s = np.einsum("khc,hc->kh", W.reshape(in_dim, HEADS, CH), att_src).astype(np.float32)
    wfold = np.zeros((in_dim, ROW), np.float32)
    wfold[:, :HC] = W
    wfold[:, HC:HC + HEADS] = wa_s
    wat = np.zeros((in_dim, 4), np.float32)
    wat[:, :HEADS] = wa_t
    w2 = (fc1_w @ fc3_w)[:, 0].astype(np.float32)      # [36]
    w2b = np.tile(w2[None, :], (P, 1))
    biasb = np.tile(bias[None, :], (P, 1))
    padrow = np.zeros((1, ROW), np.float32)
    padrow[0, HC:HC + HEADS] = PAD_A

    in_maps = []
    for c in range(N_CORES):
        node_ids = order[c::N_CORES]             # this core's dst nodes, deg-sorted
        ncnt = len(node_ids)
        idxs = np.full((P, slot_tot), pad_row_idx, np.int32)
        bidcnt = np.zeros((P, n_dst_tiles * 2), np.float32)
        xt_t = np.zeros((in_dim, n_xt_cols), np.float32)
        off = 0
        for t in range(n_dst_tiles):
            L = L_list[t]
            for i in range(P):
                k = t * P + i
                if k >= ncnt:
                    continue
                node = node_ids[k]
                s0, s1 = starts[node], ends[node]
                d = s1 - s0
                idxs[i, off:off + d] = src_sorted[s0:s1]
                bidcnt[i, 2 * t] = float(x_s_batch[node])
                bidcnt[i, 2 * t + 1] = 1.0
            off += L
        valid = min(ncnt, n_dst_tiles * P)
        xt_t[:, :valid] = x_t[node_ids[:valid]].T
        in_maps.append({
            "xs_t": xs_t, "xt_t": xt_t, "idxs": idxs, "bidcnt": bidcnt,
            "wfold": wfold, "wat": wat, "w2b": w2b, "biasb": biasb,
            "padrow": padrow,
        })

    nc = _build_nc(in_dim, n_src_tiles, n_dst_tiles, L_list, slot_tot, n_xt_cols)
    res = run_bass_kernel_spmd(nc, in_maps, core_ids=list(range(N_CORES)))

    q = np.zeros((P, 2), np.float64)
    for c in range(N_CORES):
        q += res.results[c]["q_out"]
    cnt = np.maximum(q[:, 1], 1.0)
    out = q[:, 0] / cnt
    const = float(fc1_b @ fc3_w[:, 0] + fc3_b[0])
    return (out + const).astype(np.float32)


# revision 5
# speedup vs baseline: 1.0040x; 1.0040x over previous
"""GAT (bipartite GATConv + mean-pool + 2 FC) on 8 Trainium2 NeuronCores.

Strategy: shard destination nodes across the 8 cores (each core owns N/8 dst
nodes and all edges pointing at them) so the segment softmax is fully local to
a core — no collectives needed.  Per core:

  Phase A: dense matmuls build a node table  row[n] = [h_s[n] (36) | a_s[n] (3)]
           in core-local DRAM (h_s = x_s @ W, a_s folded as x_s @ (W*att_src)),
           plus per-dst-node a_t values kept in SBUF.
  Phase B: dst nodes are processed in tiles of 128 (one node per partition,
           nodes degree-sorted so tiles have uniform run lengths).  Each node's
           incoming edges occupy L slots along its partition's free dimension;
           slot data arrives via indirect DMA row gathers from the table.
           The segment softmax (skipping the max-subtraction: logits are
           bounded, exp is safe in fp32) and the weighted message sum are
           strided DVE/ACT ops along each partition's run.  A one-hot matmul
           pools relu(out)·W2 and node counts into per-batch partials.

Host work is limited to index manipulation (edge sorting / padding / layout),
weight folding, and the final unsharding reduction of 8 x [128,2] partials.
"""

import numpy as np

import concourse.bacc as bacc
import concourse.tile as tile
from concourse import mybir
from concourse.bass import IndirectOffsetOnAxis
from concourse.bass_utils import run_bass_kernel_spmd

F32 = mybir.dt.float32
I32 = mybir.dt.int32

N_CORES = 8
P = 128
HEADS = 3
CH = 12
HC = HEADS * CH          # 36
ROW = HC + 4             # table row: 36 h | 3 a_s | 1 pad  (40 f32 = 160B)
PAD_A = -300.0           # pad-slot a_s value: exp(0.2*-300) = e^-60 ~ 0
NEG_SLOPE = 0.2

_nc_cache = {}


def _build_nc(in_dim, n_src_tiles, n_dst_tiles, L_list, slot_tot, n_xt_cols):
    """Build the SPMD Bass program (identical for all cores)."""
    key = (in_dim, n_src_tiles, n_dst_tiles, tuple(L_list), slot_tot, n_xt_cols)
    if key in _nc_cache:
        return _nc_cache[key]

    table_rows = n_src_tiles * P + 1          # +1 pad row
    pad_row_idx = n_src_tiles * P
    xs_cols = n_src_tiles * P

    nc = bacc.Bacc("TRN2", target_bir_lowering=False, debug=False)
    d_xs = nc.dram_tensor("xs_t", [in_dim, xs_cols], F32, kind="ExternalInput")
    d_xt = nc.dram_tensor("xt_t", [in_dim, n_xt_cols], F32, kind="ExternalInput")
    d_idx = nc.dram_tensor("idxs", [P, slot_tot], I32, kind="ExternalInput")
    d_bc = nc.dram_tensor("bidcnt", [P, n_dst_tiles * 2], F32, kind="ExternalInput")
    d_wf = nc.dram_tensor("wfold", [in_dim, ROW], F32, kind="ExternalInput")
    d_wt = nc.dram_tensor("wat", [in_dim, 4], F32, kind="ExternalInput")
    d_w2 = nc.dram_tensor("w2b", [P, HC], F32, kind="ExternalInput")
    d_bb = nc.dram_tensor("biasb", [P, HC], F32, kind="ExternalInput")
    d_pr = nc.dram_tensor("padrow", [1, ROW], F32, kind="ExternalInput")
    d_q = nc.dram_tensor("q_out", [P, 2], F32, kind="ExternalOutput")

    with tile.TileContext(nc) as tc:
        with tc.tile_pool(name="const", bufs=1) as cpool, \
             tc.tile_pool(name="dram", bufs=1, space="DRAM") as dpool, \
             tc.tile_pool(name="xload", bufs=2) as xpool, \
             tc.tile_pool(name="tabout", bufs=3) as topool, \
             tc.tile_pool(name="gat", bufs=6) as gpool, \
             tc.tile_pool(name="work", bufs=3) as wpool, \
             tc.tile_pool(name="psA", bufs=2, space="PSUM") as psA, \
             tc.tile_pool(name="psB", bufs=2, space="PSUM") as psB:

            table = dpool.tile([table_rows, ROW], F32)

            # ---- constants into SBUF ----
            t_wf = cpool.tile([in_dim, ROW], F32)
            nc.sync.dma_start(t_wf[:], d_wf[:])
            t_wt = cpool.tile([in_dim, 4], F32)
            nc.sync.dma_start(t_wt[:], d_wt[:])
            t_w2 = cpool.tile([P, HC], F32)
            nc.sync.dma_start(t_w2[:], d_w2[:])
            t_bb = cpool.tile([P, HC], F32)
            nc.sync.dma_start(t_bb[:], d_bb[:])
            t_pr = cpool.tile([1, ROW], F32)
            nc.sync.dma_start(t_pr[:], d_pr[:])
            t_idx = cpool.tile([P, slot_tot], I32)
            nc.sync.dma_start(t_idx[:], d_idx[:])
            t_bc = cpool.tile([P, n_dst_tiles * 2], F32)
            nc.sync.dma_start(t_bc[:], d_bc[:])
            t_xt = cpool.tile([in_dim, n_xt_cols], F32)
            nc.sync.dma_start(t_xt[:], d_xt[:])

            t_iota_i = cpool.tile([P, P], I32)
            nc.gpsimd.iota(t_iota_i[:], pattern=[[1, P]], base=0, channel_multiplier=0)
            t_iota = cpool.tile([P, P], F32)
            nc.vector.tensor_copy(t_iota[:], t_iota_i[:])

            t_qacc = cpool.tile([P, 2], F32)
            nc.vector.memset(t_qacc[:], 0.0)

            # ---- phase A: node table (h_s | a_s), batched 4 tiles / psum ----
            XB = 8  # src tiles per x-chunk load
            for blk in range(0, n_src_tiles, XB):
                nb = min(XB, n_src_tiles - blk)
                xs_sb = xpool.tile([in_dim, XB * P], F32, tag="xs")
                nc.sync.dma_start(
                    xs_sb[:, : nb * P], d_xs[:, blk * P:(blk + nb) * P])
                for g in range(0, nb, 4):
                    ng = min(4, nb - g)
                    ps = psA.tile([P, 4 * ROW], F32, space="PSUM", tag="psa")
                    for j in range(ng):
                        nc.tensor.matmul(
                            ps[:, j * ROW:(j + 1) * ROW],
                            lhsT=xs_sb[:, (g + j) * P:(g + j + 1) * P],
                            rhs=t_wf[:],
                            start=True, stop=True)
                    ob = topool.tile([P, 4 * ROW], F32, tag="tab")
                    nc.vector.tensor_copy(ob[:, : ng * ROW], ps[:, : ng * ROW])
                    # rows r = (blk+g)*128 .. +ng*128; row(p, j) = base + j*128 + p
                    base = (blk + g) * P
                    out_ap = table[:][base:base + ng * P, :]
                    # iterate (p, j, col) to match sbuf order
                    out_ap = out_ap.rearrange("(j p) c -> p j c", p=P)
                    nc.sync.dma_start(
                        out_ap,
                        ob[:, : ng * ROW].rearrange("p (j c) -> p j c", c=ROW))
            # pad row
            nc.sync.dma_start(table[:][pad_row_idx:pad_row_idx + 1, :], t_pr[:])

            # ---- phase A2: a_t per dst tile -> resident SBUF ----
            t_at = cpool.tile([P, n_dst_tiles * 4], F32)
            for t in range(n_dst_tiles):
                ps = psA.tile([P, 4], F32, space="PSUM", tag="psat")
                nc.tensor.matmul(
                    ps[:], lhsT=t_xt[:, t * P:(t + 1) * P], rhs=t_wt[:],
                    start=True, stop=True)
                nc.scalar.copy(t_at[:, t * 4:(t + 1) * 4], ps[:])

            # ---- phase B ----
            off = 0
            for t in range(n_dst_tiles):
                L = L_list[t]
                g = gpool.tile([P, L * ROW], F32, tag="G")
                for s in range(L):
                    nc.gpsimd.indirect_dma_start(
                        out=g[:, s * ROW:(s + 1) * ROW],
                        out_offset=None,
                        in_=table[:],
                        in_offset=IndirectOffsetOnAxis(
                            ap=t_idx[:, off + s:off + s + 1], axis=0),
                    )
                off += L
                g3 = g[:].rearrange("p (l c) -> p l c", c=ROW)

                # logits l = a_s + a_t  (per head, a_t per-partition scalar)
                tT = wpool.tile([P, L * HEADS], F32, tag="T")
                T3 = tT[:].rearrange("p (l h) -> p l h", h=HEADS)
                for h in range(HEADS):
                    nc.vector.tensor_scalar_add(
                        T3[:, :, h], g3[:, :, HC + h], t_at[:, t * 4 + h:t * 4 + h + 1])
                # e = exp(leaky_relu(l))
                tE = wpool.tile([P, L * HEADS], F32, tag="E")
                nc.vector.tensor_scalar_mul(tE[:], tT[:], NEG_SLOPE)
                nc.vector.tensor_tensor(
                    out=tE[:], in0=tE[:], in1=tT[:], op=mybir.AluOpType.max)
                nc.scalar.activation(tE[:], tE[:], mybir.ActivationFunctionType.Exp)
                E3 = tE[:].rearrange("p (l h) -> p l h", h=HEADS)

                # denom + reciprocal
                t_den = wpool.tile([P, HEADS], F32, tag="den")
                nc.vector.tensor_reduce(
                    out=t_den[:], in_=E3.transpose([0, 2, 1]),
                    axis=mybir.AxisListType.X, op=mybir.AluOpType.add)
                nc.vector.tensor_scalar_max(t_den[:], t_den[:], 1e-30)
                t_rec = wpool.tile([P, HEADS], F32, tag="rec")
                nc.vector.reciprocal(t_rec[:], t_den[:])

                # weighted message sum U = sum_l e * h
                tM = wpool.tile([P, L * HC], F32, tag="M")
                M3 = tM[:].rearrange("p (l j) -> p l j", j=HC)
                e_b = E3.unsqueeze(3).to_broadcast((P, L, HEADS, CH))
                nc.vector.tensor_tensor(
                    out=M3[:], in0=g3[:, :, 0:HC], in1=e_b, op=mybir.AluOpType.mult)
                tU = wpool.tile([P, HC], F32, tag="U")
                nc.vector.tensor_reduce(
                    out=tU[:], in_=M3.transpose([0, 2, 1]),
                    axis=mybir.AxisListType.X, op=mybir.AluOpType.add)

                # out = relu(U / denom + bias)
                tV = wpool.tile([P, HC], F32, tag="V")
                rec_b = t_rec[:].unsqueeze(2).to_broadcast((P, HEADS, CH))
                nc.vector.tensor_tensor(
                    out=tV[:].rearrange("p (h c) -> p h c", c=CH),
                    in0=tU[:].rearrange("p (h c) -> p h c", c=CH),
                    in1=rec_b, op=mybir.AluOpType.mult)
                nc.vector.tensor_tensor(
                    out=tV[:], in0=tV[:], in1=t_bb[:], op=mybir.AluOpType.add)
                nc.scalar.activation(tV[:], tV[:], mybir.ActivationFunctionType.Relu)

                # rv = sum(V * W2); RV = [rv | cnt]
                tR = wpool.tile([P, HC], F32, tag="R")
                nc.vector.tensor_tensor(
                    out=tR[:], in0=tV[:], in1=t_w2[:], op=mybir.AluOpType.mult)
                tRV = wpool.tile([P, 2], F32, tag="RV")
                nc.vector.tensor_reduce(
                    out=tRV[:, 0:1], in_=tR[:], axis=mybir.AxisListType.X,
                    op=mybir.AluOpType.add)
                nc.scalar.copy(tRV[:, 1:2], t_bc[:, 2 * t + 1:2 * t + 2])

                # pool into batches: q += onehot(bid)^T @ RV
                t_oh = wpool.tile([P, P], F32, tag="oh")
                nc.vector.tensor_scalar(
                    out=t_oh[:], in0=t_iota[:], scalar1=t_bc[:, 2 * t:2 * t + 1],
                    scalar2=None, op0=mybir.AluOpType.is_equal)
                ps_q = psB.tile([P, 2], F32, space="PSUM", tag="q")
                nc.tensor.matmul(ps_q[:], lhsT=t_oh[:], rhs=tRV[:],
                                 start=True, stop=True)
                nc.vector.tensor_tensor(
                    out=t_qacc[:], in0=t_qacc[:], in1=ps_q[:],
                    op=mybir.AluOpType.add)

            nc.sync.dma_start(d_q[:], t_qacc[:])
    nc.finalize()
    _nc_cache[key] = nc
    return nc


def kernel(**inputs):
    x_s = np.asarray(inputs["x_s"], np.float32)
    x_t = np.asarray(inputs["x_t"], np.float32)
    edge_index = np.asarray(inputs["edge_index"])
    x_s_batch = np.asarray(inputs["x_s_batch"]).astype(np.int64)
    W = np.asarray(inputs["W"], np.float32)
    att_src = np.asarray(inputs["att_src"], np.float32)
    att_dst = np.asarray(inputs["att_dst"], np.float32)
    bias = np.asarray(inputs["bias"], np.float32)
    fc1_w = np.asarray(inputs["fc1_w"], np.float32)
    fc1_b = np.asarray(inputs["fc1_b"], np.float32)
    fc3_w = np.asarray(inputs["fc3_w"], np.float32)
    fc3_b = np.asarray(inputs["fc3_b"], np.float32)

    n_nodes, in_dim = x_s.shape
    src = edge_index[0].astype(np.int64)
    dst = edge_index[1].astype(np.int64)

    # ---- host: edge bucketing by destination (layout prep only) ----
    deg = np.bincount(dst, minlength=n_nodes)
    order = np.argsort(-deg, kind="stable")      # nodes by degree desc
    # round-robin deal over cores: core c gets ranks c, c+8, ...
    nodes_per_core = (n_nodes + N_CORES - 1) // N_CORES
    n_dst_tiles = (nodes_per_core + P - 1) // P
    # per-tile run length: max degree in the global rank band of the tile
    L_list = []
    for t in range(n_dst_tiles):
        r0 = t * P * N_CORES
        L_list.append(max(1, int(deg[order[min(r0, n_nodes - 1)]])))
    slot_tot = int(np.sum(L_list))

    n_src_tiles = (n_nodes + P - 1) // P
    pad_row_idx = n_src_tiles * P
    xs_cols = n_src_tiles * P
    n_xt_cols = n_dst_tiles * P

    # edges sorted by dst -> per-node contiguous src runs
    e_order = np.argsort(dst, kind="stable")
    src_sorted = src[e_order].astype(np.int32)
    starts = np.searchsorted(dst[e_order], np.arange(n_nodes))
    ends = np.searchsorted(dst[e_order], np.arange(n_nodes) + 1)

    xs_t = np.zeros((in_dim, xs_cols), np.float32)
    xs_t[:, :n_nodes] = x_s.T

    # fold weights (host weight prep)
    wa_t = np.einsum("khc,hc->kh", W.reshape(in_dim, HEADS, CH), att_dst).astype(np.float32)
    wa_s = np.einsum("khc,hc->kh", W.reshape(in_dim, HEADS, CH), att_src).astype(np.float32)
    wfold = np.zeros((in_dim, ROW), np.float32)
    wfold[:, :HC] = W
    wfold[:, HC:HC + HEADS] = wa_s
    wat = np.zeros((in_dim, 4), np.float32)
    wat[:, :HEADS] = wa_t
    w2 = (fc1_w @ fc3_w)[:, 0].astype(np.float32)      # [36]
    w2b = np.tile(w2[None, :], (P, 1))
    biasb = np.tile(bias[None, :], (P, 1))
    padrow = np.zeros((1, ROW), np.float32)
    padrow[0, HC:HC + HEADS] = PAD_A

    in_maps = []
    for c in range(N_CORES):
        node_ids = order[c::N_CORES]             # this core's dst nodes, deg-sorted
        ncnt = len(node_ids)
        idxs = np.full((P, slot_tot), pad_row_idx, np.int32)
        bidcnt = np.zeros((P, n_dst_tiles * 2), np.float32)
        xt_t = np.zeros((in_dim, n_xt_cols), np.float32)
        off = 0
        for t in range(n_dst_tiles):
            L = L_list[t]
            for i in range(P):
                k = t * P + i
                if k >= ncnt:
                    continue
                node = node_ids[k]
                s0, s1 = starts[node], ends[node]
                d = s1 - s0
                idxs[i, off:off + d] = src_sorted[s0:s1]
                bidcnt[i, 2 * t] = float(x_s_batch[node])
                bidcnt[i, 2 * t + 1] = 1.0
            off += L
        valid = min(ncnt, n_dst_tiles * P)
        xt_t[:, :valid] = x_t[node_ids[:valid]].T
        in_maps.append({
            "xs_t": xs_t, "xt_t": xt_t, "idxs": idxs, "bidcnt": bidcnt,
            "wfold": wfold, "wat": wat, "w2b": w2b, "biasb": biasb,
            "padrow": padrow,
        })

    nc = _build_nc(in_dim, n_src_tiles, n_dst_tiles, L_list, slot_tot, n_xt_cols)
    res = run_bass_kernel_spmd(nc, in_maps, core_ids=list(range(N_CORES)))

    q = np.zeros((P, 2), np.float64)
    for c in range(N_CORES):
        q += res.results[c]["q_out"]
    cnt = np.maximum(q[:, 1], 1.0)
    out = q[:, 0] / cnt
    const = float(fc1_b @ fc3_w[:, 0] + fc3_b[0])
    return (out + const).astype(np.float32)


# revision 6
# speedup vs baseline: 1.0155x; 1.0115x over previous
"""GAT (bipartite GATConv + mean-pool + 2 FC) on 8 Trainium2 NeuronCores.

Strategy: shard destination nodes across the 8 cores (each core owns N/8 dst
nodes and all edges pointing at them) so the segment softmax is fully local to
a core — no collectives needed.  Per core:

  Phase A: dense matmuls build a node table  row[n] = [h_s[n] (36) | a_s[n] (3)]
           in core-local DRAM (h_s = x_s @ W, a_s folded as x_s @ (W*att_src)),
           plus per-dst-node a_t values kept in SBUF.
  Phase B: dst nodes are processed in tiles of 128 (one node per partition,
           nodes degree-sorted so tiles have uniform run lengths).  Each node's
           incoming edges occupy L slots along its partition's free dimension;
           slot data arrives via indirect DMA row gathers from the table.
           The segment softmax (skipping the max-subtraction: logits are
           bounded, exp is safe in fp32) and the weighted message sum are
           strided DVE/ACT ops along each partition's run.  A one-hot matmul
           pools relu(out)·W2 and node counts into per-batch partials.

Host work is limited to index manipulation (edge sorting / padding / layout),
weight folding, and the final unsharding reduction of 8 x [128,2] partials.
"""

import numpy as np

import concourse.bacc as bacc
import concourse.tile as tile
from concourse import mybir
from concourse.bass import IndirectOffsetOnAxis
from concourse.bass_utils import run_bass_kernel_spmd

F32 = mybir.dt.float32
I32 = mybir.dt.int32

N_CORES = 8
P = 128
HEADS = 3
CH = 12
HC = HEADS * CH          # 36
ROW = HC + 4             # table row: 36 h | 3 a_s | 1 pad  (40 f32 = 160B)
PAD_A = -300.0           # pad-slot a_s value: exp(0.2*-300) = e^-60 ~ 0
NEG_SLOPE = 0.2

_nc_cache = {}


def _build_nc(in_dim, n_src_tiles, n_dst_tiles, L_list, slot_tot, n_xt_cols):
    """Build the SPMD Bass program (identical for all cores)."""
    key = (in_dim, n_src_tiles, n_dst_tiles, tuple(L_list), slot_tot, n_xt_cols)
    if key in _nc_cache:
        return _nc_cache[key]

    table_rows = n_src_tiles * P + 1          # +1 pad row
    pad_row_idx = n_src_tiles * P
    xs_cols = n_src_tiles * P
    half_tiles = n_src_tiles // 2             # n_src_tiles forced even by caller

    nc = bacc.Bacc("TRN2", target_bir_lowering=False, debug=False)
    d_xs = nc.dram_tensor("xs_t", [2 * in_dim, xs_cols // 2], F32, kind="ExternalInput")
    d_xt = nc.dram_tensor("xt_t", [in_dim, n_xt_cols], F32, kind="ExternalInput")
    d_idx = nc.dram_tensor("idxs", [P, slot_tot], I32, kind="ExternalInput")
    d_bc = nc.dram_tensor("bidcnt", [P, n_dst_tiles * 2], F32, kind="ExternalInput")
    d_wf = nc.dram_tensor("wfold", [in_dim, ROW], F32, kind="ExternalInput")
    d_wt = nc.dram_tensor("wat", [in_dim, 4], F32, kind="ExternalInput")
    d_w2 = nc.dram_tensor("w2b", [P, HC], F32, kind="ExternalInput")
    d_bb = nc.dram_tensor("biasb", [P, HC], F32, kind="ExternalInput")
    d_pr = nc.dram_tensor("padrow", [1, ROW], F32, kind="ExternalInput")
    d_q = nc.dram_tensor("q_out", [P, 2], F32, kind="ExternalOutput")

    with tile.TileContext(nc) as tc:
        with tc.tile_pool(name="const", bufs=1) as cpool, \
             tc.tile_pool(name="dram", bufs=1, space="DRAM") as dpool, \
             tc.tile_pool(name="xload", bufs=2) as xpool, \
             tc.tile_pool(name="tabout", bufs=3) as topool, \
             tc.tile_pool(name="gat", bufs=6) as gpool, \
             tc.tile_pool(name="work", bufs=3) as wpool, \
             tc.tile_pool(name="psA", bufs=2, space="PSUM") as psA, \
             tc.tile_pool(name="psB", bufs=2, space="PSUM") as psB:

            table = dpool.tile([table_rows, ROW], F32)

            # ---- constants into SBUF ----
            t_wf = cpool.tile([in_dim, ROW], F32)
            nc.sync.dma_start(t_wf[:], d_wf[:])
            t_wf2 = cpool.tile([2 * in_dim, ROW], F32)
            nc.sync.dma_start(t_wf2[0:in_dim, :], d_wf[:])
            nc.sync.dma_start(t_wf2[in_dim:2 * in_dim, :], d_wf[:])
            t_wt = cpool.tile([in_dim, 4], F32)
            nc.sync.dma_start(t_wt[:], d_wt[:])
            t_w2 = cpool.tile([P, HC], F32)
            nc.sync.dma_start(t_w2[:], d_w2[:])
            t_bb = cpool.tile([P, HC], F32)
            nc.sync.dma_start(t_bb[:], d_bb[:])
            t_pr = cpool.tile([1, ROW], F32)
            nc.sync.dma_start(t_pr[:], d_pr[:])
            t_idx = cpool.tile([P, slot_tot], I32)
            nc.sync.dma_start(t_idx[:], d_idx[:])
            t_bc = cpool.tile([P, n_dst_tiles * 2], F32)
            nc.sync.dma_start(t_bc[:], d_bc[:])
            t_xt = cpool.tile([in_dim, n_xt_cols], F32)
            nc.sync.dma_start(t_xt[:], d_xt[:])

            t_iota_i = cpool.tile([P, P], I32)
            nc.gpsimd.iota(t_iota_i[:], pattern=[[1, P]], base=0, channel_multiplier=0)
            t_iota = cpool.tile([P, P], F32)
            nc.vector.tensor_copy(t_iota[:], t_iota_i[:])

            t_qacc = cpool.tile([P, 2], F32)
            nc.vector.memset(t_qacc[:], 0.0)

            # ---- phase A: node table (h_s | a_s) ----
            # x packed [128, half]: partitions 0:64 = tiles [0, half), 64:128 =
            # tiles [half, 2*half). Two K=64 matmuls per slice in separate PE
            # row groups; 4 tiles batched per psum bank per half.
            XB = 8  # half-tiles per x-chunk load
            for blk in range(0, half_tiles, XB):
                nb = min(XB, half_tiles - blk)
                xs_sb = xpool.tile([2 * in_dim, XB * P], F32, tag="xs")
                nc.sync.dma_start(
                    xs_sb[:, : nb * P], d_xs[:, blk * P:(blk + nb) * P])
                for g in range(0, nb, 4):
                    ng = min(4, nb - g)
                    for hf in range(2):
                        ps = psA.tile([P, 4 * ROW], F32, space="PSUM", tag="psa")
                        for j in range(ng):
                            nc.tensor.matmul(
                                ps[:, j * ROW:(j + 1) * ROW],
                                lhsT=xs_sb[hf * in_dim:(hf + 1) * in_dim,
                                           (g + j) * P:(g + j + 1) * P],
                                rhs=t_wf2[hf * in_dim:(hf + 1) * in_dim, :],
                                start=True, stop=True)
                        ob = topool.tile([P, 4 * ROW], F32, tag="tab")
                        nc.vector.tensor_copy(ob[:, : ng * ROW], ps[:, : ng * ROW])
                        base = (hf * half_tiles + blk + g) * P
                        out_ap = table[:][base:base + ng * P, :]
                        out_ap = out_ap.rearrange("(j p) c -> p j c", p=P)
                        nc.sync.dma_start(
                            out_ap,
                            ob[:, : ng * ROW].rearrange("p (j c) -> p j c", c=ROW))
            # pad row
            nc.sync.dma_start(table[:][pad_row_idx:pad_row_idx + 1, :], t_pr[:])

            # ---- phase A2: a_t per dst tile -> resident SBUF ----
            t_at = cpool.tile([P, n_dst_tiles * 4], F32)
            for t in range(n_dst_tiles):
                ps = psA.tile([P, 4], F32, space="PSUM", tag="psat")
                nc.tensor.matmul(
                    ps[:], lhsT=t_xt[:, t * P:(t + 1) * P], rhs=t_wt[:],
                    start=True, stop=True)
                nc.scalar.copy(t_at[:, t * 4:(t + 1) * 4], ps[:])

            # ---- phase B ----
            off = 0
            for t in range(n_dst_tiles):
                L = L_list[t]
                g = gpool.tile([P, L * ROW], F32, tag="G")
                for s in range(L):
                    nc.gpsimd.indirect_dma_start(
                        out=g[:, s * ROW:(s + 1) * ROW],
                        out_offset=None,
                        in_=table[:],
                        in_offset=IndirectOffsetOnAxis(
                            ap=t_idx[:, off + s:off + s + 1], axis=0),
                    )
                off += L
                g3 = g[:].rearrange("p (l c) -> p l c", c=ROW)

                # logits l = a_s + a_t  (per head, a_t per-partition scalar)
                tT = wpool.tile([P, L * HEADS], F32, tag="T")
                T3 = tT[:].rearrange("p (l h) -> p l h", h=HEADS)
                for h in range(HEADS):
                    nc.vector.tensor_scalar_add(
                        T3[:, :, h], g3[:, :, HC + h], t_at[:, t * 4 + h:t * 4 + h + 1])
                # e = exp(leaky_relu(l))
                tE = wpool.tile([P, L * HEADS], F32, tag="E")
                nc.vector.tensor_scalar_mul(tE[:], tT[:], NEG_SLOPE)
                nc.vector.tensor_tensor(
                    out=tE[:], in0=tE[:], in1=tT[:], op=mybir.AluOpType.max)
                nc.scalar.activation(tE[:], tE[:], mybir.ActivationFunctionType.Exp)
                E3 = tE[:].rearrange("p (l h) -> p l h", h=HEADS)

                # denom + reciprocal
                t_den = wpool.tile([P, HEADS], F32, tag="den")
                nc.vector.tensor_reduce(
                    out=t_den[:], in_=E3.transpose([0, 2, 1]),
                    axis=mybir.AxisListType.X, op=mybir.AluOpType.add)
                nc.vector.tensor_scalar_max(t_den[:], t_den[:], 1e-30)
                t_rec = wpool.tile([P, HEADS], F32, tag="rec")
                nc.vector.reciprocal(t_rec[:], t_den[:])

                # weighted message sum U = sum_l e * h
                tM = wpool.tile([P, L * HC], F32, tag="M")
                M3 = tM[:].rearrange("p (l j) -> p l j", j=HC)
                e_b = E3.unsqueeze(3).to_broadcast((P, L, HEADS, CH))
                nc.vector.tensor_tensor(
                    out=M3[:], in0=g3[:, :, 0:HC], in1=e_b, op=mybir.AluOpType.mult)
                tU = wpool.tile([P, HC], F32, tag="U")
                nc.vector.tensor_reduce(
                    out=tU[:], in_=M3.transpose([0, 2, 1]),
                    axis=mybir.AxisListType.X, op=mybir.AluOpType.add)

                # out = relu(U / denom + bias)
                tV = wpool.tile([P, HC], F32, tag="V")
                rec_b = t_rec[:].unsqueeze(2).to_broadcast((P, HEADS, CH))
                nc.vector.tensor_tensor(
                    out=tV[:].rearrange("p (h c) -> p h c", c=CH),
                    in0=tU[:].rearrange("p (h c) -> p h c", c=CH),
                    in1=rec_b, op=mybir.AluOpType.mult)
                nc.vector.tensor_tensor(
                    out=tV[:], in0=tV[:], in1=t_bb[:], op=mybir.AluOpType.add)
                nc.scalar.activation(tV[:], tV[:], mybir.ActivationFunctionType.Relu)

                # rv = sum(V * W2); RV = [rv | cnt]
                tR = wpool.tile([P, HC], F32, tag="R")
                nc.vector.tensor_tensor(
                    out=tR[:], in0=tV[:], in1=t_w2[:], op=mybir.AluOpType.mult)
                tRV = wpool.tile([P, 2], F32, tag="RV")
                nc.vector.tensor_reduce(
                    out=tRV[:, 0:1], in_=tR[:], axis=mybir.AxisListType.X,
                    op=mybir.AluOpType.add)
                nc.scalar.copy(tRV[:, 1:2], t_bc[:, 2 * t + 1:2 * t + 2])

                # pool into batches: q += onehot(bid)^T @ RV
                t_oh = wpool.tile([P, P], F32, tag="oh")
                nc.vector.tensor_scalar(
                    out=t_oh[:], in0=t_iota[:], scalar1=t_bc[:, 2 * t:2 * t + 1],
                    scalar2=None, op0=mybir.AluOpType.is_equal)
                ps_q = psB.tile([P, 2], F32, space="PSUM", tag="q")
                nc.tensor.matmul(ps_q[:], lhsT=t_oh[:], rhs=tRV[:],
                                 start=True, stop=True)
                nc.vector.tensor_tensor(
                    out=t_qacc[:], in0=t_qacc[:], in1=ps_q[:],
                    op=mybir.AluOpType.add)

            nc.sync.dma_start(d_q[:], t_qacc[:])
    nc.finalize()
    _nc_cache[key] = nc
    return nc


def kernel(**inputs):
    x_s = np.asarray(inputs["x_s"], np.float32)
    x_t = np.asarray(inputs["x_t"], np.float32)
    edge_index = np.asarray(inputs["edge_index"])
    x_s_batch = np.asarray(inputs["x_s_batch"]).astype(np.int64)
    W = np.asarray(inputs["W"], np.float32)
    att_src = np.asarray(inputs["att_src"], np.float32)
    att_dst = np.asarray(inputs["att_dst"], np.float32)
    bias = np.asarray(inputs["bias"], np.float32)
    fc1_w = np.asarray(inputs["fc1_w"], np.float32)
    fc1_b = np.asarray(inputs["fc1_b"], np.float32)
    fc3_w = np.asarray(inputs["fc3_w"], np.float32)
    fc3_b = np.asarray(inputs["fc3_b"], np.float32)

    n_nodes, in_dim = x_s.shape
    src = edge_index[0].astype(np.int64)
    dst = edge_index[1].astype(np.int64)

    # ---- host: edge bucketing by destination (layout prep only) ----
    deg = np.bincount(dst, minlength=n_nodes)
    order = np.argsort(-deg, kind="stable")      # nodes by degree desc
    # round-robin deal over cores: core c gets ranks c, c+8, ...
    nodes_per_core = (n_nodes + N_CORES - 1) // N_CORES
    n_dst_tiles = (nodes_per_core + P - 1) // P
    # per-tile run length: max degree in the global rank band of the tile
    L_list = []
    for t in range(n_dst_tiles):
        r0 = t * P * N_CORES
        L_list.append(max(1, int(deg[order[min(r0, n_nodes - 1)]])))
    slot_tot = int(np.sum(L_list))

    n_src_tiles = (n_nodes + P - 1) // P
    if n_src_tiles % 2:
        n_src_tiles += 1
    pad_row_idx = n_src_tiles * P
    xs_cols = n_src_tiles * P
    n_xt_cols = n_dst_tiles * P

    # edges sorted by dst -> per-node contiguous src runs
    e_order = np.argsort(dst, kind="stable")
    src_sorted = src[e_order].astype(np.int32)
    starts = np.searchsorted(dst[e_order], np.arange(n_nodes))
    ends = np.searchsorted(dst[e_order], np.arange(n_nodes) + 1)

    xs_t = np.zeros((in_dim, xs_cols), np.float32)
    xs_t[:, :n_nodes] = x_s.T
    half_cols = xs_cols // 2
    xs_t = np.concatenate([xs_t[:, :half_cols], xs_t[:, half_cols:]], axis=0)
    xs_t = np.ascontiguousarray(xs_t)

    # fold weights (host weight prep)
    wa_t = np.einsum("khc,hc->kh", W.reshape(in_dim, HEADS, CH), att_dst).astype(np.float32)
    wa_s = np.einsum("khc,hc->kh", W.reshape(in_dim, HEADS, CH), att_src).astype(np.float32)
    wfold = np.zeros((in_dim, ROW), np.float32)
    wfold[:, :HC] = W
    wfold[:, HC:HC + HEADS] = wa_s
    wat = np.zeros((in_dim, 4), np.float32)
    wat[:, :HEADS] = wa_t
    w2 = (fc1_w @ fc3_w)[:, 0].astype(np.float32)      # [36]
    w2b = np.tile(w2[None, :], (P, 1))
    biasb = np.tile(bias[None, :], (P, 1))
    padrow = np.zeros((1, ROW), np.float32)
    padrow[0, HC:HC + HEADS] = PAD_A

    in_maps = []
    for c in range(N_CORES):
        node_ids = order[c::N_CORES]             # this core's dst nodes, deg-sorted
        ncnt = len(node_ids)
        idxs = np.full((P, slot_tot), pad_row_idx, np.int32)
        bidcnt = np.zeros((P, n_dst_tiles * 2), np.float32)
        xt_t = np.zeros((in_dim, n_xt_cols), np.float32)
        off = 0
        for t in range(n_dst_tiles):
            L = L_list[t]
            for i in range(P):
                k = t * P + i
                if k >= ncnt:
                    continue
                node = node_ids[k]
                s0, s1 = starts[node], ends[node]
                d = s1 - s0
                idxs[i, off:off + d] = src_sorted[s0:s1]
                bidcnt[i, 2 * t] = float(x_s_batch[node])
                bidcnt[i, 2 * t + 1] = 1.0
            off += L
        valid = min(ncnt, n_dst_tiles * P)
        xt_t[:, :valid] = x_t[node_ids[:valid]].T
        in_maps.append({
            "xs_t": xs_t, "xt_t": xt_t, "idxs": idxs, "bidcnt": bidcnt,
            "wfold": wfold, "wat": wat, "w2b": w2b, "biasb": biasb,
            "padrow": padrow,
        })

    nc = _build_nc(in_dim, n_src_tiles, n_dst_tiles, L_list, slot_tot, n_xt_cols)
    res = run_bass_kernel_spmd(nc, in_maps, core_ids=list(range(N_CORES)))

    q = np.zeros((P, 2), np.float64)
    for c in range(N_CORES):
        q += res.results[c]["q_out"]
    cnt = np.maximum(q[:, 1], 1.0)
    out = q[:, 0] / cnt
    const = float(fc1_b @ fc3_w[:, 0] + fc3_b[0])
    return (out + const).astype(np.float32)


# revision 7
# speedup vs baseline: 1.0435x; 1.0276x over previous
"""GAT (bipartite GATConv + mean-pool + 2 FC) on 8 Trainium2 NeuronCores.

Strategy: shard destination nodes across the 8 cores (each core owns N/8 dst
nodes and all edges pointing at them) so the segment softmax is fully local to
a core — no collectives needed.  Per core:

  Phase A: dense matmuls build a node table  row[n] = [h_s[n] (36) | a_s[n] (3)]
           in core-local DRAM (h_s = x_s @ W, a_s folded as x_s @ (W*att_src)),
           plus per-dst-node a_t values kept in SBUF.
  Phase B: dst nodes are processed in tiles of 128 (one node per partition,
           nodes degree-sorted so tiles have uniform run lengths).  Each node's
           incoming edges occupy L slots along its partition's free dimension;
           slot data arrives via indirect DMA row gathers from the table.
           The segment softmax (skipping the max-subtraction: logits are
           bounded, exp is safe in fp32) and the weighted message sum are
           strided DVE/ACT ops along each partition's run.  A one-hot matmul
           pools relu(out)·W2 and node counts into per-batch partials.

Host work is limited to index manipulation (edge sorting / padding / layout),
weight folding, and the final unsharding reduction of 8 x [128,2] partials.
"""

import numpy as np

import concourse.bacc as bacc
import concourse.tile as tile
from concourse import mybir
from concourse.bass import IndirectOffsetOnAxis
from concourse.bass_utils import run_bass_kernel_spmd

F32 = mybir.dt.float32
I32 = mybir.dt.int32

N_CORES = 8
P = 128
HEADS = 3
CH = 12
HC = HEADS * CH          # 36
ROW = HC + 4             # table row: 36 h | 3 a_s | 1 pad  (40 f32 = 160B)
PAD_A = -300.0           # pad-slot a_s value: exp(0.2*-300) = e^-60 ~ 0
NEG_SLOPE = 0.2

_nc_cache = {}


def _build_nc(in_dim, n_src_tiles, n_dst_tiles, L_list, slot_tot, n_xt_cols):
    """Build the SPMD Bass program (identical for all cores)."""
    key = (in_dim, n_src_tiles, n_dst_tiles, tuple(L_list), slot_tot, n_xt_cols)
    if key in _nc_cache:
        return _nc_cache[key]

    table_rows = n_src_tiles * P + 1          # +1 pad row
    pad_row_idx = n_src_tiles * P
    xs_cols = n_src_tiles * P
    half_tiles = n_src_tiles // 2             # n_src_tiles forced even by caller

    nc = bacc.Bacc("TRN2", target_bir_lowering=False, debug=False)
    d_xs = nc.dram_tensor("xs_t", [2 * in_dim, xs_cols // 2], F32, kind="ExternalInput")
    d_xt = nc.dram_tensor("xt_t", [in_dim, n_xt_cols], F32, kind="ExternalInput")
    d_idx = nc.dram_tensor("idxs", [P, slot_tot], I32, kind="ExternalInput")
    d_bc = nc.dram_tensor("bidcnt", [P, n_dst_tiles * 2], F32, kind="ExternalInput")
    d_wf = nc.dram_tensor("wfold", [in_dim, ROW], F32, kind="ExternalInput")
    d_wt = nc.dram_tensor("wat", [in_dim, 4], F32, kind="ExternalInput")
    d_w2 = nc.dram_tensor("w2b", [P, HC], F32, kind="ExternalInput")
    d_bb = nc.dram_tensor("biasb", [P, HC], F32, kind="ExternalInput")
    d_pr = nc.dram_tensor("padrow", [1, ROW], F32, kind="ExternalInput")
    d_q = nc.dram_tensor("q_out", [P, 2], F32, kind="ExternalOutput")

    with tile.TileContext(nc) as tc:
        with tc.tile_pool(name="const", bufs=1) as cpool, \
             tc.tile_pool(name="dram", bufs=1, space="DRAM") as dpool, \
             tc.tile_pool(name="xload", bufs=3) as xpool, \
             tc.tile_pool(name="tabout", bufs=4) as topool, \
             tc.tile_pool(name="gat", bufs=6) as gpool, \
             tc.tile_pool(name="work", bufs=3) as wpool, \
             tc.tile_pool(name="psA", bufs=4, space="PSUM") as psA, \
             tc.tile_pool(name="psB", bufs=2, space="PSUM") as psB, \
             tc.tile_pool(name="psT", bufs=2, space="PSUM") as psT:

            table = dpool.tile([table_rows, ROW], F32)

            # ---- constants into SBUF ----
            t_wf = cpool.tile([in_dim, ROW], F32)
            nc.sync.dma_start(t_wf[:], d_wf[:])
            t_wf2 = cpool.tile([2 * in_dim, ROW], F32)
            nc.sync.dma_start(t_wf2[0:in_dim, :], d_wf[:])
            nc.sync.dma_start(t_wf2[in_dim:2 * in_dim, :], d_wf[:])
            t_wt = cpool.tile([in_dim, 4], F32)
            nc.sync.dma_start(t_wt[:], d_wt[:])
            t_w2 = cpool.tile([P, HC], F32)
            nc.sync.dma_start(t_w2[:], d_w2[:])
            t_bb = cpool.tile([P, HC], F32)
            nc.sync.dma_start(t_bb[:], d_bb[:])
            t_pr = cpool.tile([1, ROW], F32)
            nc.sync.dma_start(t_pr[:], d_pr[:])
            t_idx = cpool.tile([P, slot_tot], I32)
            nc.sync.dma_start(t_idx[:], d_idx[:])
            t_bc = cpool.tile([P, n_dst_tiles * 2], F32)
            nc.sync.dma_start(t_bc[:], d_bc[:])
            t_xt = cpool.tile([in_dim, n_xt_cols], F32)
            nc.sync.dma_start(t_xt[:], d_xt[:])

            t_iota_i = cpool.tile([P, P], I32)
            nc.gpsimd.iota(t_iota_i[:], pattern=[[1, P]], base=0, channel_multiplier=0)
            t_iota = cpool.tile([P, P], F32)
            nc.vector.tensor_copy(t_iota[:], t_iota_i[:])

            t_qacc = cpool.tile([P, 2], F32)
            nc.vector.memset(t_qacc[:], 0.0)

            # ---- phase A2: a_t per dst tile -> resident SBUF ----
            t_at = cpool.tile([P, n_dst_tiles * 4], F32)
            for t in range(n_dst_tiles):
                ps = psT.tile([P, 4], F32, space="PSUM", tag="psat")
                nc.tensor.matmul(
                    ps[:], lhsT=t_xt[:, t * P:(t + 1) * P], rhs=t_wt[:],
                    start=True, stop=True)
                nc.scalar.copy(t_at[:, t * 4:(t + 1) * 4], ps[:])

            # ---- phase A: node table (h_s | a_s) ----
            # x packed [128, half]: partitions 0:64 = tiles [0, half), 64:128 =
            # tiles [half, 2*half). Two K=64 matmuls per slice in separate PE
            # row groups; 4 tiles batched per psum bank per half.
            XB = 8  # half-tiles per x-chunk load
            for blk in range(0, half_tiles, XB):
                nb = min(XB, half_tiles - blk)
                xs_sb = xpool.tile([2 * in_dim, XB * P], F32, tag="xs")
                nc.sync.dma_start(
                    xs_sb[:, : nb * P], d_xs[:, blk * P:(blk + nb) * P])
                for g in range(0, nb, 4):
                    ng = min(4, nb - g)
                    for hf in range(2):
                        ps = psA.tile([P, 4 * ROW], F32, space="PSUM", tag="psa")
                        for j in range(ng):
                            nc.tensor.matmul(
                                ps[:, j * ROW:(j + 1) * ROW],
                                lhsT=xs_sb[hf * in_dim:(hf + 1) * in_dim,
                                           (g + j) * P:(g + j + 1) * P],
                                rhs=t_wf2[hf * in_dim:(hf + 1) * in_dim, :],
                                start=True, stop=True)
                        ob = topool.tile([P, 4 * ROW], F32, tag="tab")
                        nc.vector.tensor_copy(ob[:, : ng * ROW], ps[:, : ng * ROW])
                        base = (hf * half_tiles + blk + g) * P
                        out_ap = table[:][base:base + ng * P, :]
                        out_ap = out_ap.rearrange("(j p) c -> p j c", p=P)
                        nc.scalar.dma_start(
                            out_ap,
                            ob[:, : ng * ROW].rearrange("p (j c) -> p j c", c=ROW))
            # pad row
            nc.scalar.dma_start(table[:][pad_row_idx:pad_row_idx + 1, :], t_pr[:])

            # ---- phase B ----
            off = 0
            for t in range(n_dst_tiles):
                L = L_list[t]
                g = gpool.tile([P, L * ROW], F32, tag="G")
                for s in range(L):
                    nc.gpsimd.indirect_dma_start(
                        out=g[:, s * ROW:(s + 1) * ROW],
                        out_offset=None,
                        in_=table[:],
                        in_offset=IndirectOffsetOnAxis(
                            ap=t_idx[:, off + s:off + s + 1], axis=0),
                    )
                off += L
                g3 = g[:].rearrange("p (l c) -> p l c", c=ROW)

                # logits l = a_s + a_t  (per head, a_t per-partition scalar)
                tT = wpool.tile([P, L * HEADS], F32, tag="T")
                T3 = tT[:].rearrange("p (l h) -> p l h", h=HEADS)
                for h in range(HEADS):
                    nc.vector.tensor_scalar_add(
                        T3[:, :, h], g3[:, :, HC + h], t_at[:, t * 4 + h:t * 4 + h + 1])
                # e = exp(leaky_relu(l))
                tE = wpool.tile([P, L * HEADS], F32, tag="E")
                nc.vector.tensor_scalar_mul(tE[:], tT[:], NEG_SLOPE)
                nc.vector.tensor_tensor(
                    out=tE[:], in0=tE[:], in1=tT[:], op=mybir.AluOpType.max)
                nc.scalar.activation(tE[:], tE[:], mybir.ActivationFunctionType.Exp)
                E3 = tE[:].rearrange("p (l h) -> p l h", h=HEADS)

                # denom + reciprocal
                t_den = wpool.tile([P, HEADS], F32, tag="den")
                nc.vector.tensor_reduce(
                    out=t_den[:], in_=E3.transpose([0, 2, 1]),
                    axis=mybir.AxisListType.X, op=mybir.AluOpType.add)
                nc.vector.tensor_scalar_max(t_den[:], t_den[:], 1e-30)
                t_rec = wpool.tile([P, HEADS], F32, tag="rec")
                nc.vector.reciprocal(t_rec[:], t_den[:])

                # weighted message sum U = sum_l e * h
                tM = wpool.tile([P, L * HC], F32, tag="M")
                M3 = tM[:].rearrange("p (l j) -> p l j", j=HC)
                e_b = E3.unsqueeze(3).to_broadcast((P, L, HEADS, CH))
                nc.vector.tensor_tensor(
                    out=M3[:], in0=g3[:, :, 0:HC], in1=e_b, op=mybir.AluOpType.mult)
                tU = wpool.tile([P, HC], F32, tag="U")
                nc.vector.tensor_reduce(
                    out=tU[:], in_=M3.transpose([0, 2, 1]),
                    axis=mybir.AxisListType.X, op=mybir.AluOpType.add)

                # out = relu(U / denom + bias)
                tV = wpool.tile([P, HC], F32, tag="V")
                rec_b = t_rec[:].unsqueeze(2).to_broadcast((P, HEADS, CH))
                nc.vector.tensor_tensor(
                    out=tV[:].rearrange("p (h c) -> p h c", c=CH),
                    in0=tU[:].rearrange("p (h c) -> p h c", c=CH),
                    in1=rec_b, op=mybir.AluOpType.mult)
                nc.vector.tensor_tensor(
                    out=tV[:], in0=tV[:], in1=t_bb[:], op=mybir.AluOpType.add)
                nc.scalar.activation(tV[:], tV[:], mybir.ActivationFunctionType.Relu)

                # rv = sum(V * W2); RV = [rv | cnt]
                tR = wpool.tile([P, HC], F32, tag="R")
                nc.vector.tensor_tensor(
                    out=tR[:], in0=tV[:], in1=t_w2[:], op=mybir.AluOpType.mult)
                tRV = wpool.tile([P, 2], F32, tag="RV")
                nc.vector.tensor_reduce(
                    out=tRV[:, 0:1], in_=tR[:], axis=mybir.AxisListType.X,
                    op=mybir.AluOpType.add)
                nc.scalar.copy(tRV[:, 1:2], t_bc[:, 2 * t + 1:2 * t + 2])

                # pool into batches: q += onehot(bid)^T @ RV
                t_oh = wpool.tile([P, P], F32, tag="oh")
                nc.vector.tensor_scalar(
                    out=t_oh[:], in0=t_iota[:], scalar1=t_bc[:, 2 * t:2 * t + 1],
                    scalar2=None, op0=mybir.AluOpType.is_equal)
                ps_q = psB.tile([P, 2], F32, space="PSUM", tag="q")
                nc.tensor.matmul(ps_q[:], lhsT=t_oh[:], rhs=tRV[:],
                                 start=True, stop=True)
                nc.vector.tensor_tensor(
                    out=t_qacc[:], in0=t_qacc[:], in1=ps_q[:],
                    op=mybir.AluOpType.add)

            nc.sync.dma_start(d_q[:], t_qacc[:])
    nc.finalize()
    _nc_cache[key] = nc
    return nc


def kernel(**inputs):
    x_s = np.asarray(inputs["x_s"], np.float32)
    x_t = np.asarray(inputs["x_t"], np.float32)
    edge_index = np.asarray(inputs["edge_index"])
    x_s_batch = np.asarray(inputs["x_s_batch"]).astype(np.int64)
    W = np.asarray(inputs["W"], np.float32)
    att_src = np.asarray(inputs["att_src"], np.float32)
    att_dst = np.asarray(inputs["att_dst"], np.float32)
    bias = np.asarray(inputs["bias"], np.float32)
    fc1_w = np.asarray(inputs["fc1_w"], np.float32)
    fc1_b = np.asarray(inputs["fc1_b"], np.float32)
    fc3_w = np.asarray(inputs["fc3_w"], np.float32)
    fc3_b = np.asarray(inputs["fc3_b"], np.float32)

    n_nodes, in_dim = x_s.shape
    src = edge_index[0].astype(np.int64)
    dst = edge_index[1].astype(np.int64)

    # ---- host: edge bucketing by destination (layout prep only) ----
    deg = np.bincount(dst, minlength=n_nodes)
    order = np.argsort(-deg, kind="stable")      # nodes by degree desc
    # round-robin deal over cores: core c gets ranks c, c+8, ...
    nodes_per_core = (n_nodes + N_CORES - 1) // N_CORES
    n_dst_tiles = (nodes_per_core + P - 1) // P
    # per-tile run length: max degree in the global rank band of the tile
    L_list = []
    for t in range(n_dst_tiles):
        r0 = t * P * N_CORES
        L_list.append(max(1, int(deg[order[min(r0, n_nodes - 1)]])))
    slot_tot = int(np.sum(L_list))

    n_src_tiles = (n_nodes + P - 1) // P
    if n_src_tiles % 2:
        n_src_tiles += 1
    pad_row_idx = n_src_tiles * P
    xs_cols = n_src_tiles * P
    n_xt_cols = n_dst_tiles * P

    # edges sorted by dst -> per-node contiguous src runs
    e_order = np.argsort(dst, kind="stable")
    src_sorted = src[e_order].astype(np.int32)
    starts = np.searchsorted(dst[e_order], np.arange(n_nodes))
    ends = np.searchsorted(dst[e_order], np.arange(n_nodes) + 1)

    xs_t = np.zeros((in_dim, xs_cols), np.float32)
    xs_t[:, :n_nodes] = x_s.T
    half_cols = xs_cols // 2
    xs_t = np.concatenate([xs_t[:, :half_cols], xs_t[:, half_cols:]], axis=0)
    xs_t = np.ascontiguousarray(xs_t)

    # fold weights (host weight prep)
    wa_t = np.einsum("khc,hc->kh", W.reshape(in_dim, HEADS, CH), att_dst).astype(np.float32)
    wa_s = np.einsum("khc,hc->kh", W.reshape(in_dim, HEADS, CH), att_src).astype(np.float32)
    wfold = np.zeros((in_dim, ROW), np.float32)
    wfold[:, :HC] = W
    wfold[:, HC:HC + HEADS] = wa_s
    wat = np.zeros((in_dim, 4), np.float32)
    wat[:, :HEADS] = wa_t
    w2 = (fc1_w @ fc3_w)[:, 0].astype(np.float32)      # [36]
    w2b = np.tile(w2[None, :], (P, 1))
    biasb = np.tile(bias[None, :], (P, 1))
    padrow = np.zeros((1, ROW), np.float32)
    padrow[0, HC:HC + HEADS] = PAD_A

    in_maps = []
    for c in range(N_CORES):
        node_ids = order[c::N_CORES]             # this core's dst nodes, deg-sorted
        ncnt = len(node_ids)
        idxs = np.full((P, slot_tot), pad_row_idx, np.int32)
        bidcnt = np.zeros((P, n_dst_tiles * 2), np.float32)
        xt_t = np.zeros((in_dim, n_xt_cols), np.float32)
        off = 0
        for t in range(n_dst_tiles):
            L = L_list[t]
            for i in range(P):
                k = t * P + i
                if k >= ncnt:
                    continue
                node = node_ids[k]
                s0, s1 = starts[node], ends[node]
                d = s1 - s0
                idxs[i, off:off + d] = src_sorted[s0:s1]
                bidcnt[i, 2 * t] = float(x_s_batch[node])
                bidcnt[i, 2 * t + 1] = 1.0
            off += L
        valid = min(ncnt, n_dst_tiles * P)
        xt_t[:, :valid] = x_t[node_ids[:valid]].T
        in_maps.append({
            "xs_t": xs_t, "xt_t": xt_t, "idxs": idxs, "bidcnt": bidcnt,
            "wfold": wfold, "wat": wat, "w2b": w2b, "biasb": biasb,
            "padrow": padrow,
        })

    nc = _build_nc(in_dim, n_src_tiles, n_dst_tiles, L_list, slot_tot, n_xt_cols)
    res = run_bass_kernel_spmd(nc, in_maps, core_ids=list(range(N_CORES)))

    q = np.zeros((P, 2), np.float64)
    for c in range(N_CORES):
        q += res.results[c]["q_out"]
    cnt = np.maximum(q[:, 1], 1.0)
    out = q[:, 0] / cnt
    const = float(fc1_b @ fc3_w[:, 0] + fc3_b[0])
    return (out + const).astype(np.float32)
